# revision 5
# baseline (speedup 1.0000x reference)
"""Trainium2 Bass kernel for nn_EpipolarWarpOperator (B=8, C=320, H=W=64, S=3).

Sharding: spatial — every core computes an 8-row horizontal strip of all 8
batch outputs (the 3x3 conv is strip-separable; epipolar sampling per strip
with a 1-row halo).

Host analysis classifies each batch by its epipolar sampling map:

* pattern batches: the per-pixel bilinear sampling signature map has few
  distinct values, so the sampled image is piecewise constant and the conv
  output has <= npat distinct "pattern" columns whose column-group structure
  ([edge | interior | edge] per row) is fixed across rows and cores. The
  device computes R[k, tap, pi] from a host-gathered slab via tiny matmuls,
  T[pi, ch] = sum R_tap^T W_tap, then a per-core permutation matmul
  T2E[ch, slot] (slot = (batch, colgroup, striprow)) with relu+bias on the
  drain, and finally paints the output strip with a handful of broadcast
  copies on DVE/Pool/ACT (no per-pixel PE work).

* dense batches: the whole image touches only a few hundred distinct bilinear
  corner pixels; the host gathers them into <=NSL global slabs [128, C] plus
  per-window sampling matrices; the device runs swapped-operand matmuls
  accumulating all slabs into PSUM (two windows: 8 rows + 2 halo rows),
  then a 24-matmul-per-mc 3x3 conv over the 10-row sampled plane with
  bias+relu on the drain.
"""

import numpy as np

B, C, H, W = 8, 320, 64, 64
SN = 3
HW = H * W
STRIP = 8             # output rows per core
NCORE = 8
MB = [(0, 128), (128, 128), (256, 64)]   # output/input channel chunking
PI_MAX = 126          # total pattern budget
NPAT_MAX = 56         # per-batch pattern cap
NSIG_MAX = 64
SW = STRIP * W        # 512 px per strip

# ---------------------------------------------------------------- host: geometry


def _rodrigues_np(rv):
    theta = np.sqrt((rv * rv).sum())
    r = rv / max(theta, 1e-12)
    I = np.eye(3, dtype=np.float32)
    K = np.array([[0, -r[2], r[1]], [r[2], 0, -r[0]], [-r[1], r[0], 0]],
                 dtype=np.float32)
    R = np.cos(theta) * I + (1 - np.cos(theta)) * np.outer(r, r) + np.sin(theta) * K
    return I if theta < 1e-6 else R


def fundamental_np(Ks, Kt, ps, pt):
    Fs = []
    for b in range(Ks.shape[0]):
        Rs = _rodrigues_np(ps[b, :3].astype(np.float32))
        Rt = _rodrigues_np(pt[b, :3].astype(np.float32))
        ts_, tt_ = ps[b, 3:].astype(np.float32), pt[b, 3:].astype(np.float32)
        R_rel = Rs @ Rt.T
        t_rel = ts_ - R_rel @ tt_
        z = np.float32(0)
        skew = np.array([[z, -t_rel[2], t_rel[1]],
                         [t_rel[2], z, -t_rel[0]],
                         [-t_rel[1], t_rel[0], z]], dtype=np.float32)
        E = skew @ R_rel
        inv_Ks = np.linalg.inv(Ks[b].astype(np.float32))
        inv_Kt = np.linalg.inv(Kt[b].astype(np.float32))
        Fs.append(inv_Kt.T @ E @ inv_Ks)
    return np.stack(Fs).astype(np.float32)


def geometry(F):
    k = np.arange(HW)
    px = (k % W).astype(np.float32)
    py = (k // W).astype(np.float32)
    P = np.stack([px, py, np.ones_like(px)])
    lines = F.T.astype(np.float32) @ P
    a, b_, c = lines[0], lines[1], lines[2]
    W1, H1 = np.float32(W - 1), np.float32(H - 1)
    EPS = np.float32(1e-10)
    x1 = np.clip(-c / (a + EPS), 0.0, W1)
    x2 = np.clip(-(b_ * H1 + c) / (a + EPS), 0.0, W1)
    y1 = np.clip(-c / (b_ + EPS), 0.0, H1)
    y2 = np.clip(-(a * W1 + c) / (b_ + EPS), 0.0, H1)
    t = np.linspace(0.0, 1.0, SN, dtype=np.float32)
    sx = x1[:, None] * (1 - t) + x2[:, None] * t
    sy = y1[:, None] * (1 - t) + y2[:, None] * t
    x0 = np.floor(sx)
    y0 = np.floor(sy)
    wx = (sx - x0).astype(np.float32)
    wy = (sy - y0).astype(np.float32)
    x0i = np.clip(x0, 0, W - 1).astype(np.int32)
    y0i = np.clip(y0, 0, H - 1).astype(np.int32)
    return x0i, y0i, wx, wy


def _corners(geo, p, s):
    """4 bilinear (source pixel row, weight/3) pairs for pixel p, sample s."""
    x0i, y0i, wx, wy = geo
    y0 = int(y0i[p, s]); x0 = int(x0i[p, s])
    x1 = min(x0 + 1, W - 1); y1 = min(y0 + 1, H - 1)
    wxx = np.float32(wx[p, s]); wyy = np.float32(wy[p, s])
    third = np.float32(1.0 / 3.0)
    out = {}
    for ry, rx, ww in ((y0, x0, (1 - wxx) * (1 - wyy)),
                       (y0, x1, wxx * (1 - wyy)),
                       (y1, x0, (1 - wxx) * wyy),
                       (y1, x1, wxx * wyy)):
        rr = ry * W + rx
        out[rr] = out.get(rr, np.float32(0)) + ww * third
    return out


# ------------------------------------------------------------- host: classify


def classify(geo):
    x0i, y0i, wx, wy = geo
    key = np.concatenate([
        y0i.astype(np.float32), x0i.astype(np.float32), wx, wy], axis=1)
    kview = np.ascontiguousarray(key).view([('', key.dtype)] * key.shape[1]).ravel()
    uniq, first, inv = np.unique(kview, return_index=True, return_inverse=True)
    if len(uniq) > NSIG_MAX:
        return None
    tables = []
    for si in range(len(uniq)):
        p = int(first[si])
        contrib = {}
        for s in range(SN):
            for rr, ww in _corners(geo, p, s).items():
                contrib[rr] = contrib.get(rr, np.float32(0)) + ww
        tables.append(sorted(contrib.items()))
    return inv.astype(np.int32), tables


def patterns(siginv):
    """3x3 neighborhood patterns of the signature map (border = -1)."""
    simg = siginv.reshape(H, W)
    pad = np.pad(simg, 1, constant_values=-1)
    neigh = np.stack([pad[dy:dy + H, dx:dx + W].ravel()
                      for dy in range(3) for dx in range(3)], axis=1)
    uniq, pinv = np.unique(neigh, axis=0, return_inverse=True)
    return pinv.astype(np.int32), uniq


def col_groups(pinv):
    """Fixed column-group partition of the pattern map: boundaries where any
    row changes pattern. Returns ((off, len), ...)."""
    pimg = pinv.reshape(H, W)
    bset = set()
    for r in range(H):
        row = pimg[r]
        for c in np.where(row[1:] != row[:-1])[0]:
            bset.add(int(c))
    bounds = sorted(bset)
    groups = []
    start = 0
    for b in bounds:
        groups.append((start, b + 1 - start))
        start = b + 1
    groups.append((start, W - start))
    return tuple(groups)


# ------------------------------------------------------------ host: build plan


def _deg_slabs(entries):
    """Assign signatures to slabs so each slab's source-pixel union <= 128."""
    slabs = []
    rows, sigs = {}, []
    for bloc, si, tab in entries:
        new = [(bloc, rr) for rr, _ in tab if (bloc, rr) not in rows]
        if len(rows) + len(new) > 128:
            slabs.append((rows, sigs))
            rows, sigs = {}, []
            new = [(bloc, rr) for rr, _ in tab]
        for key in new:
            rows[key] = len(rows)
        sigs.append((bloc, si))
    slabs.append((rows, sigs))
    return slabs


def _dense_sources(geo, r0, r1):
    """Sorted unique corner source rows for dest rows [r0, r1) (clipped)."""
    x0i, y0i = geo[0], geo[1]
    s = set()
    for r in range(max(r0, 0), min(r1, H)):
        for cx in range(W):
            p = r * W + cx
            for sm in range(SN):
                y0, x0 = int(y0i[p, sm]), int(x0i[p, sm])
                x1, y1 = min(x0 + 1, W - 1), min(y0 + 1, H - 1)
                s.add(y0 * W + x0); s.add(y0 * W + x1)
                s.add(y1 * W + x0); s.add(y1 * W + x1)
    return sorted(s)


def make_plan(x, source_intrinsics, target_intrinsics, source_pose,
              target_pose, conv_w, conv_b):
    Fs = fundamental_np(np.asarray(source_intrinsics, np.float32),
                        np.asarray(target_intrinsics, np.float32),
                        np.asarray(source_pose, np.float32),
                        np.asarray(target_pose, np.float32))
    x = np.asarray(x, np.float32)
    xT16 = [np.ascontiguousarray(x[b].reshape(C, HW).T).astype(np.float16)
            for b in range(B)]

    degs, denses = [], []
    pi_used = 0
    for b in range(B):
        geo = geometry(Fs[b])
        res = classify(geo)
        if res is not None:
            siginv, tables = res
            pinv, ptab = patterns(siginv)
            npat = ptab.shape[0]
            if npat <= NPAT_MAX and pi_used + npat <= PI_MAX:
                degs.append(dict(gb=b, siginv=siginv, tables=tables,
                                 pinv=pinv, ptab=ptab, pi_off=pi_used,
                                 groups=col_groups(pinv)))
                pi_used += npat
                continue
        denses.append(dict(gb=b, geo=geo))
    pi_tot = pi_used

    # ---- degenerate global data (same for all cores) ----
    RGROUP_CAP = 512
    rgroups = []
    cur, cur_n = [], 0
    for bi, d in enumerate(degs):
        n9 = 9 * d['ptab'].shape[0]
        if cur and cur_n + n9 > RGROUP_CAP:
            rgroups.append(dict(bis=cur, ncols=cur_n))
            cur, cur_n = [], 0
        cur.append(bi)
        cur_n += n9
    if cur:
        rgroups.append(dict(bis=cur, ncols=cur_n))

    sg_blocks, sr_blocks = [], []
    for g in rgroups:
        entries = []
        for bloc, bi in enumerate(g['bis']):
            for si, tab in enumerate(degs[bi]['tables']):
                entries.append((bloc, si, tab))
        slabs = _deg_slabs(entries)
        g['nslab'] = len(slabs)
        g['q0'] = degs[g['bis'][0]]['pi_off']
        g['nq'] = sum(degs[bi]['ptab'].shape[0] for bi in g['bis'])
        nq = g['nq']
        for rows, sigs in slabs:
            slab = np.zeros((128, C), dtype=np.float16)
            for (bloc, rr), idx in rows.items():
                slab[idx] = xT16[degs[g['bis'][bloc]]['gb']][rr]
            SR = np.zeros((128, g['ncols']), dtype=np.float32)
            sigset = set(sigs)
            for bloc, bi in enumerate(g['bis']):
                d = degs[bi]
                ptab = d['ptab']
                qb = d['pi_off'] - g['q0']
                for pi in range(ptab.shape[0]):
                    for tap in range(9):
                        si = ptab[pi, tap]
                        if si >= 0 and (bloc, si) in sigset:
                            for rr, ww in d['tables'][si]:
                                SR[rows[(bloc, rr)],
                                   tap * nq + qb + pi] += ww
            sg_blocks.append(slab)
            sr_blocks.append(SR.astype(np.float16))
    sg = (np.concatenate(sg_blocks, axis=1) if sg_blocks
          else np.zeros((128, 0), np.float16))
    sr = (np.concatenate(sr_blocks, axis=1) if sr_blocks
          else np.zeros((128, 0), np.float16))

    # slot layout for the permutation matmul: per deg batch, per colgroup,
    # per strip row (group-major so paint sources are slot-contiguous)
    slot_base = []
    nslot = 0
    for d in degs:
        slot_base.append(nslot)
        nslot += STRIP * len(d['groups'])

    # ---- dense per-core slabs & window sampling matrices ----
    # windows: A = strip rows -1..6 (8 rows, 512 px), B = rows 7..8 (2 rows)
    WINS = ((-1, 8), (7, 2))
    nsl = 1
    core_src = []
    for r in range(NCORE):
        srcs = []
        for d in denses:
            sl = _dense_sources(d['geo'], 8 * r - 1, 8 * r + 9)
            srcs.append(sl)
            nsl = max(nsl, (len(sl) + 127) // 128)
        core_src.append(srcs)
    NSL = nsl

    # ---- weights ----
    Wl = np.zeros((128, 3 * 9 * C), dtype=np.float16)
    cw = np.asarray(conv_w, np.float32)
    cb = np.asarray(conv_b, np.float32)
    for kc, (koff, ksz) in enumerate(MB):
        for tap in range(9):
            dy, dx = tap // 3, tap % 3
            Wl[0:ksz, kc * 9 * C + tap * C: kc * 9 * C + tap * C + C] = \
                cw[:, koff:koff + ksz, dy, dx].T.astype(np.float16)
    # paired kc=2 weights: partitions 0:64 = dy=0 tap, 64:128 = dy=1 tap
    # (read through the +1-row shifted duplicate of the kc=2 sampled plane)
    Wl2 = np.zeros((128, 3 * C), dtype=np.float16)
    for dxi in range(3):
        Wl2[0:64, dxi * C: dxi * C + C] = \
            cw[:, 256:320, 0, dxi].T.astype(np.float16)
        Wl2[64:128, dxi * C: dxi * C + C] = \
            cw[:, 256:320, 1, dxi].T.astype(np.float16)
    bias = np.zeros((128, 3), dtype=np.float32)
    for mc, (moff, msz) in enumerate(MB):
        bias[0:msz, mc] = cb[moff:moff + msz]

    # ---- per-core in_maps ----
    ndeg, ndense = len(degs), len(denses)
    slots = [d['gb'] for d in degs] + [d['gb'] for d in denses]
    in_maps = []
    for r in range(NCORE):
        m = {"wl": Wl, "wl2": Wl2, "bias": bias}
        if ndeg:
            m["sg"] = sg
            m["sr"] = sr
            # permutation matrix: P[pi, slot] with slot = base + g*8 + row
            P = np.zeros((128, nslot), dtype=np.float16)
            for bi, d in enumerate(degs):
                pimg = d['pinv'].reshape(H, W)
                for gi, (off, _ln) in enumerate(d['groups']):
                    for row in range(STRIP):
                        pi = pimg[8 * r + row, off]
                        P[d['pi_off'] + pi,
                          slot_base[bi] + gi * STRIP + row] = 1.0
            m["p"] = P
        if ndense:
            sds, sss = [], []
            for di, d in enumerate(denses):
                sl = core_src[r][di]
                rowmap = {rr: i for i, rr in enumerate(sl)}
                slab = np.zeros((128 * NSL, C), dtype=np.float16)
                if sl:
                    slab[:len(sl)] = xT16[d['gb']][np.array(sl)]
                sds.append(slab.reshape(NSL, 128, C).transpose(1, 0, 2)
                           .reshape(128, NSL * C))
                Sw = []
                for w0, wn in WINS:
                    Smat = np.zeros((128 * NSL, wn * W), dtype=np.float32)
                    for lr in range(wn):
                        row = 8 * r + w0 + lr
                        if not (0 <= row < H):
                            continue
                        for cx in range(W):
                            p = row * W + cx
                            for sm in range(SN):
                                for rr, ww in _corners(d['geo'], p, sm).items():
                                    Smat[rowmap[rr], lr * W + cx] += ww
                    Sw.append(Smat.astype(np.float16))
                # layout: [128, NSL*512 (win A) + NSL*128 (win B)]
                a = Sw[0].reshape(NSL, 128, 8 * W).transpose(1, 0, 2)
                bmat = Sw[1].reshape(NSL, 128, 2 * W).transpose(1, 0, 2)
                sss.append(np.concatenate(
                    [a.reshape(128, -1), bmat.reshape(128, -1)], axis=1))
            m["sd"] = np.concatenate(sds, axis=1)
            m["ss"] = np.concatenate(sss, axis=1)
        in_maps.append(m)

    struct = (pi_tot, nslot, NSL,
              tuple((d['gb'], d['ptab'].shape[0], d['groups'])
                    for d in degs),
              tuple((g['ncols'], g['nslab'], g['q0'], g['nq'])
                    for g in rgroups),
              tuple(d['gb'] for d in denses))
    return in_maps, struct, slots


# ------------------------------------------------------------- bass program

_NC_CACHE = {}


def build_program(reps, struct):
    key = (reps, struct)
    if key in _NC_CACHE:
        return _NC_CACHE[key]
    import concourse.bacc as bacc
    import concourse.mybir as mybir
    from concourse.tile import TileContext

    fp16 = mybir.dt.float16
    f32 = mybir.dt.float32
    pi_tot, nslot, NSL, degs, rgroups, dense_gbs = struct
    ndeg, ndense = len(degs), len(dense_gbs)
    NB = ndeg + ndense
    NSG = sum(ns for _, ns, _, _ in rgroups)
    SRC = sum(nc_ * ns for nc_, ns, _, _ in rgroups)

    nc = bacc.Bacc(target_bir_lowering=False)
    wl_d = nc.dram_tensor("wl", [128, 3 * 9 * C], fp16, kind="ExternalInput")
    wl2_d = nc.dram_tensor("wl2", [128, 3 * C], fp16, kind="ExternalInput")
    bias_d = nc.dram_tensor("bias", [128, 3], f32, kind="ExternalInput")
    if ndeg:
        sg_d = nc.dram_tensor("sg", [128, NSG * C], fp16, kind="ExternalInput")
        sr_d = nc.dram_tensor("sr", [128, SRC], fp16, kind="ExternalInput")
        p_d = nc.dram_tensor("p", [128, nslot], fp16, kind="ExternalInput")
    if ndense:
        sd_d = nc.dram_tensor("sd", [128, ndense * NSL * C], fp16,
                              kind="ExternalInput")
        ss_d = nc.dram_tensor("ss", [128, ndense * NSL * 640], fp16,
                              kind="ExternalInput")
    ob01_d = nc.dram_tensor("ob01", [128, NB * 2 * SW], fp16,
                            kind="ExternalOutput")
    ob2_d = nc.dram_tensor("ob2", [64, NB * SW], fp16, kind="ExternalOutput")

    with TileContext(nc) as tc:
        with tc.tile_pool(name="const", bufs=1) as constp, \
             tc.tile_pool(name="inp", bufs=2) as inp, \
             tc.tile_pool(name="sdp", bufs=2) as sdp, \
             tc.tile_pool(name="work", bufs=2) as work, \
             tc.tile_pool(name="smpp", bufs=2) as smpp, \
             tc.tile_pool(name="outp", bufs=2) as outp, \
             tc.tile_pool(name="psA", bufs=4, space="PSUM") as psA, \
             tc.tile_pool(name="psB", bufs=4, space="PSUM") as psB:
            wl = constp.tile([128, 3 * 9 * C], fp16)
            nc.sync.dma_start(out=wl[:], in_=wl_d[:])
            wl2 = constp.tile([128, 3 * C], fp16)
            nc.sync.dma_start(out=wl2[:], in_=wl2_d[:])
            bias_t = constp.tile([128, 3], f32)
            nc.sync.dma_start(out=bias_t[:], in_=bias_d[:])

            def body(_it):
                ob01 = outp.tile([128, NB, 2, SW], fp16, name="ob01",
                                 tag="ob01")
                ob2 = outp.tile([64, NB, SW], fp16, name="ob2", tag="ob2")

                # ---------- input DMAs ----------
                if ndeg:
                    sg = inp.tile([128, NSG * C], fp16, name="sg", tag="sg")
                    nc.sync.dma_start(out=sg[:], in_=sg_d[:])
                    sr = inp.tile([128, SRC], fp16, name="sr", tag="sr")
                    nc.sync.dma_start(out=sr[:], in_=sr_d[:])
                    p_sb = inp.tile([128, nslot], fp16, name="p", tag="p")
                    nc.sync.dma_start(out=p_sb[0:pi_tot, :],
                                      in_=p_d[0:pi_tot, :])
                if ndense:
                    sd = sdp.tile([128, ndense * NSL * C], fp16, name="sd",
                                  tag="sd")
                    nc.sync.dma_start(out=sd[:], in_=sd_d[:])
                    ss = sdp.tile([128, ndense * NSL * 640], fp16, name="ss",
                                  tag="ss")
                    nc.sync.dma_start(out=ss[:], in_=ss_d[:])

                # ---------- degenerate path: R ----------
                if ndeg:
                    rsb = work.tile([128, 3, 9, pi_tot], fp16, name="rsb",
                                    tag="rsb")
                    sgo, sro = 0, 0
                    for gi, (ncols, nsl, q0, nq) in enumerate(rgroups):
                        for kc, (koff, ksz) in enumerate(MB):
                            ps_r = psB.tile([128, 512], f32,
                                            name=f"psr{gi}_{kc}", tag="psB")
                            for j in range(nsl):
                                nc.tensor.matmul(
                                    ps_r[0:ksz, 0:ncols],
                                    sg[:, (sgo + j) * C + koff:
                                       (sgo + j) * C + koff + ksz],
                                    sr[:, sro + j * ncols:
                                       sro + (j + 1) * ncols],
                                    start=(j == 0), stop=(j == nsl - 1))
                            psq = ps_r[:, 0:ncols].rearrange(
                                "p (t q) -> p t q", t=9)
                            if kc % 2 == 0:
                                nc.vector.tensor_copy(
                                    rsb[0:ksz, kc, :, q0:q0 + nq], psq[0:ksz])
                            else:
                                nc.scalar.copy(
                                    rsb[0:ksz, kc, :, q0:q0 + nq], psq[0:ksz])
                        sgo += nsl
                        sro += nsl * ncols

                # ---------- dense sampling ----------
                smps = []
                if ndense:
                    for di in range(ndense):
                        # plane row i = image row 8r-1+i; cols 1..64 = image
                        smp = smpp.tile([128, 3, 10, 66], fp16,
                                        name=f"smp{di}", tag=f"smp{di}")
                        nc.gpsimd.memset(smp[:, :, :, 0:1], 0.0)
                        nc.gpsimd.memset(smp[:, :, :, 65:66], 0.0)
                        smps.append(smp)

                    for di in range(ndense):
                        smp = smps[di]
                        sdo = di * NSL * C
                        sso = di * NSL * 640
                        # window A: 8 rows, 512 px
                        pws = []
                        for kc, (koff, ksz) in enumerate(MB):
                            pw = psA.tile([128, 512], f32,
                                          name=f"pswA{di}_{kc}", tag="psA")
                            for j in range(NSL):
                                nc.tensor.matmul(
                                    pw[0:ksz, :],
                                    sd[:, sdo + j * C + koff:
                                       sdo + j * C + koff + ksz],
                                    ss[:, sso + j * 512:sso + (j + 1) * 512],
                                    start=(j == 0), stop=(j == NSL - 1))
                            pws.append(pw)
                        # window B: 2 rows, 128 px (3 kc share one bank)
                        pwb = psA.tile([128, 512], f32, name=f"pswB{di}",
                                       tag="psA")
                        for kc, (koff, ksz) in enumerate(MB):
                            for j in range(NSL):
                                nc.tensor.matmul(
                                    pwb[0:ksz, kc * 128:(kc + 1) * 128],
                                    sd[:, sdo + j * C + koff:
                                       sdo + j * C + koff + ksz],
                                    ss[:, sso + NSL * 512 + j * 128:
                                       sso + NSL * 512 + (j + 1) * 128],
                                    start=(j == 0), stop=(j == NSL - 1))
                        # drains: win A rows 0..7, win B rows 8..9
                        for kc, (koff, ksz) in enumerate(MB):
                            pv = pws[kc][:, :].rearrange("p (r c) -> p r c",
                                                         r=8)
                            if kc == 1:
                                nc.scalar.copy(smp[0:ksz, kc, 0:8, 1:65],
                                               pv[0:ksz])
                            else:
                                nc.vector.tensor_copy(
                                    smp[0:ksz, kc, 0:8, 1:65], pv[0:ksz])
                        pvb = pwb[:, 0:384].rearrange("p (k r c) -> p k r c",
                                                      k=3, r=2)
                        nc.vector.tensor_copy(smp[0:128, 0:2, 8:10, 1:65],
                                              pvb[0:128, 0:2])
                        nc.scalar.copy(smp[0:64, 2, 8:10, 1:65],
                                       pvb[0:64, 2])
                        # +1-row shifted duplicate of kc=2 plane in parts
                        # 64:128 (for the paired dy0/dy1 conv taps)
                        nc.gpsimd.tensor_copy(smp[64:128, 2, 0:9, 1:65],
                                               smp[0:64, 2, 1:10, 1:65])

                # ---------- degenerate path: T, perm, paint ----------
                if ndeg:
                    ps_t = psB.tile([128, 512], f32, name="ps_t", tag="psB")
                    k = 0
                    for kc, (koff, ksz) in enumerate(MB):
                        for tap in range(9):
                            nc.tensor.matmul(
                                ps_t[0:pi_tot, 0:C],
                                rsb[0:ksz, kc, tap, :],
                                wl[0:ksz, kc * 9 * C + tap * C:
                                   kc * 9 * C + tap * C + C],
                                start=(k == 0), stop=(k == 26))
                            k += 1
                    tsb = work.tile([128, C], fp16, name="tsb", tag="tsb")
                    nc.scalar.copy(tsb[0:pi_tot, :], ps_t[0:pi_tot, 0:C])

                    t2e = work.tile([128, 3, nslot], fp16, name="t2e",
                                    tag="t2e")
                    for mc, (moff, msz) in enumerate(MB):
                        ps_e = psB.tile([128, 512], f32, name=f"pse{mc}",
                                        tag="psB")
                        nc.tensor.matmul(
                            ps_e[0:msz, 0:nslot],
                            tsb[0:pi_tot, moff:moff + msz],
                            p_sb[0:pi_tot, :],
                            start=True, stop=True)
                        nc.scalar.activation(
                            t2e[0:msz, mc, :], ps_e[0:msz, 0:nslot],
                            mybir.ActivationFunctionType.Relu,
                            bias=bias_t[0:msz, mc:mc + 1])

                    # paint strips: per (batch, colgroup): broadcast copies
                    paint_eng = [nc.vector.tensor_copy,
                                 nc.gpsimd.tensor_copy,
                                 nc.scalar.copy]
                    ei = 0
                    for bi, (gb, npat, groups) in enumerate(degs):
                        base = sum(STRIP * len(g3[2]) for g3 in degs[:bi])
                        for gi, (off, ln) in enumerate(groups):
                            s0 = base + gi * STRIP
                            src01 = t2e[0:128, 0:2, s0:s0 + STRIP]
                            src2 = t2e[0:64, 2, s0:s0 + STRIP]
                            d01 = ob01[0:128, bi, :, :].rearrange(
                                "p m (r c) -> p m r c", r=STRIP)
                            d2 = ob2[0:64, bi, :].rearrange(
                                "p (r c) -> p r c", r=STRIP)
                            cp = paint_eng[ei % 3]
                            ei += 1
                            cp2 = paint_eng[ei % 3]
                            ei += 1
                            if ln == 1:
                                cp(d01[:, :, :, off], src01)
                                cp2(d2[:, :, off], src2)
                            else:
                                # loop order [p, m, len, row] so the source's
                                # last dim is packed (2x/4x DVE modes)
                                dstv = d01[:, :, :, off:off + ln].transpose(
                                    [0, 1, 3, 2])
                                srcv = src01.unsqueeze(2).broadcast_to(
                                    [128, 2, ln, STRIP])
                                cp(dstv, srcv)
                                dstv2 = d2[:, :, off:off + ln].transpose(
                                    [0, 2, 1])
                                srcv2 = src2.unsqueeze(1).broadcast_to(
                                    [64, ln, STRIP])
                                cp2(dstv2, srcv2)

                # ---------- dense conv ----------
                for di in range(ndense):
                    smp = smps[di]
                    for mc, (moff, msz) in enumerate(MB):
                        ps_c = psB.tile([128, 512], f32,
                                        name=f"psc{di}_{mc}", tag="psB")
                        k = 0
                        N_MM = 24
                        for kc, (koff, ksz) in enumerate(MB[:2]):
                            for tap in range(9):
                                dy, dx = tap // 3, tap % 3
                                nc.tensor.matmul(
                                    ps_c[0:msz, :],
                                    wl[0:ksz,
                                       kc * 9 * C + tap * C + moff:
                                       kc * 9 * C + tap * C + moff + msz],
                                    smp[0:ksz, kc, dy:dy + 8, dx:dx + 64],
                                    start=(k == 0), stop=(k == N_MM - 1))
                                k += 1
                        for dx in range(3):
                            # paired kc=2 taps dy=0 (parts 0:64) and dy=1
                            # (parts 64:128, shifted duplicate)
                            nc.tensor.matmul(
                                ps_c[0:msz, :],
                                wl2[0:128, dx * C + moff:
                                    dx * C + moff + msz],
                                smp[0:128, 2, 0:8, dx:dx + 64],
                                start=False, stop=(k == N_MM - 1))
                            k += 1
                            # single kc=2 tap dy=2
                            nc.tensor.matmul(
                                ps_c[0:msz, :],
                                wl[0:64,
                                   2 * 9 * C + (6 + dx) * C + moff:
                                   2 * 9 * C + (6 + dx) * C + moff + msz],
                                smp[0:64, 2, 2:10, dx:dx + 64],
                                start=False, stop=(k == N_MM - 1))
                            k += 1
                        if mc < 2:
                            dst = ob01[0:msz, ndeg + di, mc, :]
                        else:
                            dst = ob2[0:msz, ndeg + di, :]
                        if mc == 1:
                            nc.vector.tensor_scalar(
                                dst, ps_c[0:msz, :],
                                bias_t[0:msz, mc:mc + 1], 0.0,
                                mybir.AluOpType.add, mybir.AluOpType.max)
                        else:
                            nc.scalar.activation(
                                dst, ps_c[0:msz, :],
                                mybir.ActivationFunctionType.Relu,
                                bias=bias_t[0:msz, mc:mc + 1])

                # ---------- output DMAs ----------
                nc.sync.dma_start(out=ob01_d[:, :], in_=ob01[:, :, :, :])
                nc.sync.dma_start(out=ob2_d[:, :], in_=ob2[:, :, :])

            if reps == 1:
                body(0)
            else:
                U = 4
                n_loop = reps // U
                hints = (mybir.EngineType.PE, mybir.EngineType.Activation,
                         mybir.EngineType.Pool, mybir.EngineType.SP,
                         mybir.EngineType.DVE)
                with tc.For_i(0, n_loop, 1, hint_engines=hints) as it:
                    for u in range(U):
                        body(u)
                for u in range(reps - n_loop * U):
                    body(u)

    nc.finalize()
    _NC_CACHE[key] = nc
    return nc


# ---------------------------------------------------------------- interface


def make_in_maps(x, source_intrinsics, target_intrinsics, source_pose,
                 target_pose, conv_w, conv_b):
    return make_plan(x, source_intrinsics, target_intrinsics, source_pose,
                     target_pose, conv_w, conv_b)


def assemble(results, slots):
    """results: per-core {"ob01": [128, NB*2*SW], "ob2": [64, NB*SW]}."""
    out = np.zeros((B, C, H, W), dtype=np.float32)
    NBl = len(slots)
    for r in range(NCORE):
        o01 = np.asarray(results[r]["ob01"]).reshape(128, NBl, 2, STRIP, W)
        o2 = np.asarray(results[r]["ob2"]).reshape(64, NBl, STRIP, W)
        for si, gb in enumerate(slots):
            out[gb, 0:128, 8 * r: 8 * r + STRIP, :] = \
                o01[:, si, 0].astype(np.float32)
            out[gb, 128:256, 8 * r: 8 * r + STRIP, :] = \
                o01[:, si, 1].astype(np.float32)
            out[gb, 256:320, 8 * r: 8 * r + STRIP, :] = \
                o2[:, si].astype(np.float32)
    return out


def kernel(x, source_intrinsics, target_intrinsics, source_pose,
           target_pose, conv_w, conv_b, _reps=1):
    from concourse.bass_utils import run_bass_kernel_spmd
    in_maps, struct, slots = make_in_maps(
        x, source_intrinsics, target_intrinsics, source_pose,
        target_pose, conv_w, conv_b)
    nc = build_program(_reps, struct)
    res = run_bass_kernel_spmd(nc, in_maps, list(range(NCORE)))
    return assemble(res.results, slots)


# revision 6
# speedup vs baseline: 1.4199x; 1.4199x over previous
"""Trainium2 Bass kernel for nn_EpipolarWarpOperator (B=8, C=320, H=W=64, S=3).

Sharding: spatial — every core computes an 8-row horizontal strip of all 8
batch outputs (the 3x3 conv is strip-separable; epipolar sampling per strip
with a 1-row halo).

Host analysis classifies each batch by its epipolar sampling map:

* pattern batches: the per-pixel bilinear sampling signature map has few
  distinct values, so the sampled image is piecewise constant and the conv
  output has <= npat distinct "pattern" columns whose column-group structure
  ([edge | interior | edge] per row) is fixed across rows and cores. The
  device computes R[k, tap, pi] from a host-gathered slab via tiny matmuls,
  T[pi, ch] = sum R_tap^T W_tap, then a per-core permutation matmul
  T2E[ch, slot] (slot = (batch, colgroup, striprow)) with relu+bias on the
  drain, and finally paints the output strip with a handful of broadcast
  copies on DVE/Pool/ACT (no per-pixel PE work).

* dense batches: the whole image touches only a few hundred distinct bilinear
  corner pixels; the host gathers them into <=NSL global slabs [128, C] plus
  per-window sampling matrices; the device runs swapped-operand matmuls
  accumulating all slabs into PSUM (two windows: 8 rows + 2 halo rows),
  then a 24-matmul-per-mc 3x3 conv over the 10-row sampled plane with
  bias+relu on the drain.
"""

import numpy as np

B, C, H, W = 8, 320, 64, 64
SN = 3
HW = H * W
STRIP = 8             # output rows per core
NCORE = 8
MB = [(0, 128), (128, 128), (256, 64)]   # output/input channel chunking
PI_MAX = 126          # total pattern budget
NPAT_MAX = 56         # per-batch pattern cap
NSIG_MAX = 64
SW = STRIP * W        # 512 px per strip

# ---------------------------------------------------------------- host: geometry


def _rodrigues_np(rv):
    theta = np.sqrt((rv * rv).sum())
    r = rv / max(theta, 1e-12)
    I = np.eye(3, dtype=np.float32)
    K = np.array([[0, -r[2], r[1]], [r[2], 0, -r[0]], [-r[1], r[0], 0]],
                 dtype=np.float32)
    R = np.cos(theta) * I + (1 - np.cos(theta)) * np.outer(r, r) + np.sin(theta) * K
    return I if theta < 1e-6 else R


def fundamental_np(Ks, Kt, ps, pt):
    Fs = []
    for b in range(Ks.shape[0]):
        Rs = _rodrigues_np(ps[b, :3].astype(np.float32))
        Rt = _rodrigues_np(pt[b, :3].astype(np.float32))
        ts_, tt_ = ps[b, 3:].astype(np.float32), pt[b, 3:].astype(np.float32)
        R_rel = Rs @ Rt.T
        t_rel = ts_ - R_rel @ tt_
        z = np.float32(0)
        skew = np.array([[z, -t_rel[2], t_rel[1]],
                         [t_rel[2], z, -t_rel[0]],
                         [-t_rel[1], t_rel[0], z]], dtype=np.float32)
        E = skew @ R_rel
        inv_Ks = np.linalg.inv(Ks[b].astype(np.float32))
        inv_Kt = np.linalg.inv(Kt[b].astype(np.float32))
        Fs.append(inv_Kt.T @ E @ inv_Ks)
    return np.stack(Fs).astype(np.float32)


def geometry(F):
    k = np.arange(HW)
    px = (k % W).astype(np.float32)
    py = (k // W).astype(np.float32)
    P = np.stack([px, py, np.ones_like(px)])
    lines = F.T.astype(np.float32) @ P
    a, b_, c = lines[0], lines[1], lines[2]
    W1, H1 = np.float32(W - 1), np.float32(H - 1)
    EPS = np.float32(1e-10)
    x1 = np.clip(-c / (a + EPS), 0.0, W1)
    x2 = np.clip(-(b_ * H1 + c) / (a + EPS), 0.0, W1)
    y1 = np.clip(-c / (b_ + EPS), 0.0, H1)
    y2 = np.clip(-(a * W1 + c) / (b_ + EPS), 0.0, H1)
    t = np.linspace(0.0, 1.0, SN, dtype=np.float32)
    sx = x1[:, None] * (1 - t) + x2[:, None] * t
    sy = y1[:, None] * (1 - t) + y2[:, None] * t
    x0 = np.floor(sx)
    y0 = np.floor(sy)
    wx = (sx - x0).astype(np.float32)
    wy = (sy - y0).astype(np.float32)
    x0i = np.clip(x0, 0, W - 1).astype(np.int32)
    y0i = np.clip(y0, 0, H - 1).astype(np.int32)
    return x0i, y0i, wx, wy


def _corners(geo, p, s):
    """4 bilinear (source pixel row, weight/3) pairs for pixel p, sample s."""
    x0i, y0i, wx, wy = geo
    y0 = int(y0i[p, s]); x0 = int(x0i[p, s])
    x1 = min(x0 + 1, W - 1); y1 = min(y0 + 1, H - 1)
    wxx = np.float32(wx[p, s]); wyy = np.float32(wy[p, s])
    third = np.float32(1.0 / 3.0)
    out = {}
    for ry, rx, ww in ((y0, x0, (1 - wxx) * (1 - wyy)),
                       (y0, x1, wxx * (1 - wyy)),
                       (y1, x0, (1 - wxx) * wyy),
                       (y1, x1, wxx * wyy)):
        rr = ry * W + rx
        out[rr] = out.get(rr, np.float32(0)) + ww * third
    return out


# ------------------------------------------------------------- host: classify


def classify(geo):
    x0i, y0i, wx, wy = geo
    key = np.concatenate([
        y0i.astype(np.float32), x0i.astype(np.float32), wx, wy], axis=1)
    kview = np.ascontiguousarray(key).view([('', key.dtype)] * key.shape[1]).ravel()
    uniq, first, inv = np.unique(kview, return_index=True, return_inverse=True)
    if len(uniq) > NSIG_MAX:
        return None
    tables = []
    for si in range(len(uniq)):
        p = int(first[si])
        contrib = {}
        for s in range(SN):
            for rr, ww in _corners(geo, p, s).items():
                contrib[rr] = contrib.get(rr, np.float32(0)) + ww
        tables.append(sorted(contrib.items()))
    return inv.astype(np.int32), tables


def patterns(siginv):
    """3x3 neighborhood patterns of the signature map (border = -1)."""
    simg = siginv.reshape(H, W)
    pad = np.pad(simg, 1, constant_values=-1)
    neigh = np.stack([pad[dy:dy + H, dx:dx + W].ravel()
                      for dy in range(3) for dx in range(3)], axis=1)
    uniq, pinv = np.unique(neigh, axis=0, return_inverse=True)
    return pinv.astype(np.int32), uniq


def col_groups(pinv):
    """Fixed column-group partition of the pattern map: boundaries where any
    row changes pattern. Returns ((off, len), ...)."""
    pimg = pinv.reshape(H, W)
    bset = set()
    for r in range(H):
        row = pimg[r]
        for c in np.where(row[1:] != row[:-1])[0]:
            bset.add(int(c))
    bounds = sorted(bset)
    groups = []
    start = 0
    for b in bounds:
        groups.append((start, b + 1 - start))
        start = b + 1
    groups.append((start, W - start))
    return tuple(groups)


# ------------------------------------------------------------ host: build plan


def _deg_slabs(entries):
    """Assign signatures to slabs so each slab's source-pixel union <= 128."""
    slabs = []
    rows, sigs = {}, []
    for bloc, si, tab in entries:
        new = [(bloc, rr) for rr, _ in tab if (bloc, rr) not in rows]
        if len(rows) + len(new) > 128:
            slabs.append((rows, sigs))
            rows, sigs = {}, []
            new = [(bloc, rr) for rr, _ in tab]
        for key in new:
            rows[key] = len(rows)
        sigs.append((bloc, si))
    slabs.append((rows, sigs))
    return slabs


def _dense_sources(geo, r0, r1):
    """Sorted unique corner source rows for dest rows [r0, r1) (clipped)."""
    x0i, y0i = geo[0], geo[1]
    s = set()
    for r in range(max(r0, 0), min(r1, H)):
        for cx in range(W):
            p = r * W + cx
            for sm in range(SN):
                y0, x0 = int(y0i[p, sm]), int(x0i[p, sm])
                x1, y1 = min(x0 + 1, W - 1), min(y0 + 1, H - 1)
                s.add(y0 * W + x0); s.add(y0 * W + x1)
                s.add(y1 * W + x0); s.add(y1 * W + x1)
    return sorted(s)


def make_plan(x, source_intrinsics, target_intrinsics, source_pose,
              target_pose, conv_w, conv_b):
    Fs = fundamental_np(np.asarray(source_intrinsics, np.float32),
                        np.asarray(target_intrinsics, np.float32),
                        np.asarray(source_pose, np.float32),
                        np.asarray(target_pose, np.float32))
    x = np.asarray(x, np.float32)
    xT16 = [np.ascontiguousarray(x[b].reshape(C, HW).T).astype(np.float16)
            for b in range(B)]

    degs, denses = [], []
    pi_used = 0
    for b in range(B):
        geo = geometry(Fs[b])
        res = classify(geo)
        if res is not None:
            siginv, tables = res
            pinv, ptab = patterns(siginv)
            npat = ptab.shape[0]
            if npat <= NPAT_MAX and pi_used + npat <= PI_MAX:
                degs.append(dict(gb=b, siginv=siginv, tables=tables,
                                 pinv=pinv, ptab=ptab, pi_off=pi_used,
                                 groups=col_groups(pinv)))
                pi_used += npat
                continue
        denses.append(dict(gb=b, geo=geo))
    pi_tot = pi_used

    # ---- degenerate global data (same for all cores) ----
    RGROUP_CAP = 512
    rgroups = []
    cur, cur_n = [], 0
    for bi, d in enumerate(degs):
        n9 = 9 * d['ptab'].shape[0]
        if cur and cur_n + n9 > RGROUP_CAP:
            rgroups.append(dict(bis=cur, ncols=cur_n))
            cur, cur_n = [], 0
        cur.append(bi)
        cur_n += n9
    if cur:
        rgroups.append(dict(bis=cur, ncols=cur_n))

    sg_blocks, sr_blocks = [], []
    for g in rgroups:
        entries = []
        for bloc, bi in enumerate(g['bis']):
            for si, tab in enumerate(degs[bi]['tables']):
                entries.append((bloc, si, tab))
        slabs = _deg_slabs(entries)
        g['nslab'] = len(slabs)
        g['q0'] = degs[g['bis'][0]]['pi_off']
        g['nq'] = sum(degs[bi]['ptab'].shape[0] for bi in g['bis'])
        nq = g['nq']
        for rows, sigs in slabs:
            slab = np.zeros((128, C), dtype=np.float16)
            for (bloc, rr), idx in rows.items():
                slab[idx] = xT16[degs[g['bis'][bloc]]['gb']][rr]
            SR = np.zeros((128, g['ncols']), dtype=np.float32)
            sigset = set(sigs)
            for bloc, bi in enumerate(g['bis']):
                d = degs[bi]
                ptab = d['ptab']
                qb = d['pi_off'] - g['q0']
                for pi in range(ptab.shape[0]):
                    for tap in range(9):
                        si = ptab[pi, tap]
                        if si >= 0 and (bloc, si) in sigset:
                            for rr, ww in d['tables'][si]:
                                SR[rows[(bloc, rr)],
                                   tap * nq + qb + pi] += ww
            sg_blocks.append(slab)
            sr_blocks.append(SR.astype(np.float16))
    sg = (np.concatenate(sg_blocks, axis=1) if sg_blocks
          else np.zeros((128, 0), np.float16))
    sr = (np.concatenate(sr_blocks, axis=1) if sr_blocks
          else np.zeros((128, 0), np.float16))

    # slot layout for the permutation matmul: per deg batch, per colgroup,
    # per strip row (group-major so paint sources are slot-contiguous)
    slot_base = []
    nslot = 0
    for d in degs:
        slot_base.append(nslot)
        nslot += STRIP * len(d['groups'])

    # ---- dense per-core slabs & window sampling matrices ----
    # windows: A = strip rows -1..6 (8 rows, 512 px), B = rows 7..8 (2 rows)
    WINS = ((-1, 8), (7, 2))
    nsl = 1
    core_src = []
    for r in range(NCORE):
        srcs = []
        for d in denses:
            sl = _dense_sources(d['geo'], 8 * r - 1, 8 * r + 9)
            srcs.append(sl)
            nsl = max(nsl, (len(sl) + 127) // 128)
        core_src.append(srcs)
    NSL = nsl

    # ---- weights ----
    Wl = np.zeros((128, 3 * 9 * C), dtype=np.float16)
    cw = np.asarray(conv_w, np.float32)
    cb = np.asarray(conv_b, np.float32)
    for kc, (koff, ksz) in enumerate(MB):
        for tap in range(9):
            dy, dx = tap // 3, tap % 3
            Wl[0:ksz, kc * 9 * C + tap * C: kc * 9 * C + tap * C + C] = \
                cw[:, koff:koff + ksz, dy, dx].T.astype(np.float16)
    # paired kc=2 weights: partitions 0:64 = dy=0 tap, 64:128 = dy=1 tap
    # (read through the +1-row shifted duplicate of the kc=2 sampled plane)
    Wl2 = np.zeros((128, 3 * C), dtype=np.float16)
    for dxi in range(3):
        Wl2[0:64, dxi * C: dxi * C + C] = \
            cw[:, 256:320, 0, dxi].T.astype(np.float16)
        Wl2[64:128, dxi * C: dxi * C + C] = \
            cw[:, 256:320, 1, dxi].T.astype(np.float16)
    bias = np.zeros((128, 3), dtype=np.float32)
    for mc, (moff, msz) in enumerate(MB):
        bias[0:msz, mc] = cb[moff:moff + msz]

    # ---- per-core in_maps ----
    ndeg, ndense = len(degs), len(denses)
    slots = [d['gb'] for d in degs] + [d['gb'] for d in denses]
    in_maps = []
    for r in range(NCORE):
        m = {"wl": Wl, "wl2": Wl2, "bias": bias}
        if ndeg:
            m["sg"] = sg
            m["sr"] = sr
            # permutation matrix: P[pi, slot] with slot = base + g*8 + row
            P = np.zeros((128, nslot), dtype=np.float16)
            for bi, d in enumerate(degs):
                pimg = d['pinv'].reshape(H, W)
                for gi, (off, _ln) in enumerate(d['groups']):
                    for row in range(STRIP):
                        pi = pimg[8 * r + row, off]
                        P[d['pi_off'] + pi,
                          slot_base[bi] + gi * STRIP + row] = 1.0
            m["p"] = P
        if ndense:
            sds, sss = [], []
            for di, d in enumerate(denses):
                sl = core_src[r][di]
                rowmap = {rr: i for i, rr in enumerate(sl)}
                slab = np.zeros((128 * NSL, C), dtype=np.float16)
                if sl:
                    slab[:len(sl)] = xT16[d['gb']][np.array(sl)]
                sds.append(slab.reshape(NSL, 128, C).transpose(1, 0, 2)
                           .reshape(128, NSL * C))
                Sw = []
                for w0, wn in WINS:
                    Smat = np.zeros((128 * NSL, wn * W), dtype=np.float32)
                    for lr in range(wn):
                        row = 8 * r + w0 + lr
                        if not (0 <= row < H):
                            continue
                        for cx in range(W):
                            p = row * W + cx
                            for sm in range(SN):
                                for rr, ww in _corners(d['geo'], p, sm).items():
                                    Smat[rowmap[rr], lr * W + cx] += ww
                    Sw.append(Smat.astype(np.float16))
                # layout: [128, NSL*512 (win A) + NSL*128 (win B)]
                a = Sw[0].reshape(NSL, 128, 8 * W).transpose(1, 0, 2)
                bmat = Sw[1].reshape(NSL, 128, 2 * W).transpose(1, 0, 2)
                sss.append(np.concatenate(
                    [a.reshape(128, -1), bmat.reshape(128, -1)], axis=1))
            m["sd"] = np.concatenate(sds, axis=1)
            m["ss"] = np.concatenate(sss, axis=1)
        in_maps.append(m)

    struct = (pi_tot, nslot, NSL,
              tuple((d['gb'], d['ptab'].shape[0], d['groups'])
                    for d in degs),
              tuple((g['ncols'], g['nslab'], g['q0'], g['nq'])
                    for g in rgroups),
              tuple(d['gb'] for d in denses))
    return in_maps, struct, slots


# ------------------------------------------------------------- bass program

_NC_CACHE = {}


def build_program(reps, struct):
    key = (reps, struct)
    if key in _NC_CACHE:
        return _NC_CACHE[key]
    import concourse.bacc as bacc
    import concourse.mybir as mybir
    from concourse.tile import TileContext

    fp16 = mybir.dt.float16
    f32 = mybir.dt.float32
    pi_tot, nslot, NSL, degs, rgroups, dense_gbs = struct
    ndeg, ndense = len(degs), len(dense_gbs)
    NB = ndeg + ndense
    NSG = sum(ns for _, ns, _, _ in rgroups)
    SRC = sum(nc_ * ns for nc_, ns, _, _ in rgroups)

    nc = bacc.Bacc(target_bir_lowering=False)
    wl_d = nc.dram_tensor("wl", [128, 3 * 9 * C], fp16, kind="ExternalInput")
    wl2_d = nc.dram_tensor("wl2", [128, 3 * C], fp16, kind="ExternalInput")
    bias_d = nc.dram_tensor("bias", [128, 3], f32, kind="ExternalInput")
    if ndeg:
        sg_d = nc.dram_tensor("sg", [128, NSG * C], fp16, kind="ExternalInput")
        sr_d = nc.dram_tensor("sr", [128, SRC], fp16, kind="ExternalInput")
        p_d = nc.dram_tensor("p", [128, nslot], fp16, kind="ExternalInput")
    if ndense:
        sd_d = nc.dram_tensor("sd", [128, ndense * NSL * C], fp16,
                              kind="ExternalInput")
        ss_d = nc.dram_tensor("ss", [128, ndense * NSL * 640], fp16,
                              kind="ExternalInput")
    ob01_d = nc.dram_tensor("ob01", [128, NB * 2 * SW], fp16,
                            kind="ExternalOutput")
    ob2_d = nc.dram_tensor("ob2", [64, NB * SW], fp16, kind="ExternalOutput")

    with TileContext(nc) as tc:
        with tc.tile_pool(name="const", bufs=1) as constp, \
             tc.tile_pool(name="inp", bufs=2) as inp, \
             tc.tile_pool(name="sdp", bufs=2) as sdp, \
             tc.tile_pool(name="work", bufs=2) as work, \
             tc.tile_pool(name="smpp", bufs=2) as smpp, \
             tc.tile_pool(name="outp", bufs=2) as outp, \
             tc.tile_pool(name="psA", bufs=4, space="PSUM") as psA, \
             tc.tile_pool(name="psB", bufs=4, space="PSUM") as psB:
            wl = constp.tile([128, 3 * 9 * C], fp16)
            nc.sync.dma_start(out=wl[:], in_=wl_d[:])
            wl2 = constp.tile([128, 3 * C], fp16)
            nc.sync.dma_start(out=wl2[:], in_=wl2_d[:])
            bias_t = constp.tile([128, 3], f32)
            nc.sync.dma_start(out=bias_t[:], in_=bias_d[:])

            def body(_it):
                ob01 = outp.tile([128, NB, 2, SW], fp16, name="ob01",
                                 tag="ob01")
                ob2 = outp.tile([64, NB, SW], fp16, name="ob2", tag="ob2")

                # ---------- input DMAs ----------
                if ndeg:
                    sg = inp.tile([128, NSG * C], fp16, name="sg", tag="sg")
                    nc.sync.dma_start(out=sg[:], in_=sg_d[:])
                    sr = inp.tile([128, SRC], fp16, name="sr", tag="sr")
                    nc.sync.dma_start(out=sr[:], in_=sr_d[:])
                    p_sb = inp.tile([128, nslot], fp16, name="p", tag="p")
                    nc.sync.dma_start(out=p_sb[0:pi_tot, :],
                                      in_=p_d[0:pi_tot, :])
                if ndense:
                    sd = sdp.tile([128, ndense * NSL * C], fp16, name="sd",
                                  tag="sd")
                    nc.sync.dma_start(out=sd[:], in_=sd_d[:])
                    ss = sdp.tile([128, ndense * NSL * 640], fp16, name="ss",
                                  tag="ss")
                    nc.sync.dma_start(out=ss[:], in_=ss_d[:])

                # ---------- degenerate path: R ----------
                if ndeg:
                    rsb = work.tile([128, 3, 9, pi_tot], fp16, name="rsb",
                                    tag="rsb")
                    sgo, sro = 0, 0
                    for gi, (ncols, nsl, q0, nq) in enumerate(rgroups):
                        for kc, (koff, ksz) in enumerate(MB):
                            ps_r = psB.tile([128, 512], f32,
                                            name=f"psr{gi}_{kc}", tag="psB")
                            for j in range(nsl):
                                nc.tensor.matmul(
                                    ps_r[0:ksz, 0:ncols],
                                    sg[:, (sgo + j) * C + koff:
                                       (sgo + j) * C + koff + ksz],
                                    sr[:, sro + j * ncols:
                                       sro + (j + 1) * ncols],
                                    start=(j == 0), stop=(j == nsl - 1))
                            psq = ps_r[:, 0:ncols].rearrange(
                                "p (t q) -> p t q", t=9)
                            if kc % 2 == 0:
                                nc.vector.tensor_copy(
                                    rsb[0:ksz, kc, :, q0:q0 + nq], psq[0:ksz])
                            else:
                                nc.scalar.copy(
                                    rsb[0:ksz, kc, :, q0:q0 + nq], psq[0:ksz])
                        sgo += nsl
                        sro += nsl * ncols

                # ---------- dense sampling ----------
                smps = []
                if ndense:
                    for di in range(ndense):
                        # plane row i = image row 8r-1+i; cols 1..64 = image
                        smp = smpp.tile([128, 3, 10, 66], fp16,
                                        name=f"smp{di}", tag=f"smp{di}")
                        nc.gpsimd.memset(smp[:, :, :, 0:1], 0.0)
                        nc.gpsimd.memset(smp[:, :, :, 65:66], 0.0)
                        smps.append(smp)

                    for di in range(ndense):
                        smp = smps[di]
                        sdo = di * NSL * C
                        sso = di * NSL * 640
                        # window A: 8 rows, 512 px
                        pws = []
                        for kc, (koff, ksz) in enumerate(MB):
                            pw = psA.tile([128, 512], f32,
                                          name=f"pswA{di}_{kc}", tag="psA")
                            for j in range(NSL):
                                nc.tensor.matmul(
                                    pw[0:ksz, :],
                                    sd[:, sdo + j * C + koff:
                                       sdo + j * C + koff + ksz],
                                    ss[:, sso + j * 512:sso + (j + 1) * 512],
                                    start=(j == 0), stop=(j == NSL - 1))
                            pws.append(pw)
                        # window B: 2 rows, 128 px (3 kc share one bank)
                        pwb = psA.tile([128, 512], f32, name=f"pswB{di}",
                                       tag="psA")
                        for kc, (koff, ksz) in enumerate(MB):
                            for j in range(NSL):
                                nc.tensor.matmul(
                                    pwb[0:ksz, kc * 128:(kc + 1) * 128],
                                    sd[:, sdo + j * C + koff:
                                       sdo + j * C + koff + ksz],
                                    ss[:, sso + NSL * 512 + j * 128:
                                       sso + NSL * 512 + (j + 1) * 128],
                                    start=(j == 0), stop=(j == NSL - 1))
                        # drains: win A rows 0..7, win B rows 8..9
                        for kc, (koff, ksz) in enumerate(MB):
                            pv = pws[kc][:, :].rearrange("p (r c) -> p r c",
                                                         r=8)
                            if kc == 1:
                                nc.scalar.copy(smp[0:ksz, kc, 0:8, 1:65],
                                               pv[0:ksz])
                            else:
                                nc.vector.tensor_copy(
                                    smp[0:ksz, kc, 0:8, 1:65], pv[0:ksz])
                        pvb = pwb[:, 0:384].rearrange("p (k r c) -> p k r c",
                                                      k=3, r=2)
                        nc.vector.tensor_copy(smp[0:128, 0:2, 8:10, 1:65],
                                              pvb[0:128, 0:2])
                        nc.scalar.copy(smp[0:64, 2, 8:10, 1:65],
                                       pvb[0:64, 2])
                        # +1-row shifted duplicate of kc=2 plane in parts
                        # 64:128 (for the paired dy0/dy1 conv taps)
                        nc.gpsimd.tensor_copy(smp[64:128, 2, 0:9, 1:65],
                                               smp[0:64, 2, 1:10, 1:65])

                # ---------- degenerate path: T, perm, paint ----------
                if ndeg:
                    ps_t = psB.tile([128, 512], f32, name="ps_t", tag="psB")
                    k = 0
                    for kc, (koff, ksz) in enumerate(MB):
                        for tap in range(9):
                            nc.tensor.matmul(
                                ps_t[0:pi_tot, 0:C],
                                rsb[0:ksz, kc, tap, :],
                                wl[0:ksz, kc * 9 * C + tap * C:
                                   kc * 9 * C + tap * C + C],
                                start=(k == 0), stop=(k == 26))
                            k += 1
                    tsb = work.tile([128, C], fp16, name="tsb", tag="tsb")
                    nc.scalar.copy(tsb[0:pi_tot, :], ps_t[0:pi_tot, 0:C])

                    t2e = work.tile([128, 3, nslot], fp16, name="t2e",
                                    tag="t2e")
                    for mc, (moff, msz) in enumerate(MB):
                        ps_e = psB.tile([128, 512], f32, name=f"pse{mc}",
                                        tag="psB")
                        nc.tensor.matmul(
                            ps_e[0:msz, 0:nslot],
                            tsb[0:pi_tot, moff:moff + msz],
                            p_sb[0:pi_tot, :],
                            start=True, stop=True)
                        nc.scalar.activation(
                            t2e[0:msz, mc, :], ps_e[0:msz, 0:nslot],
                            mybir.ActivationFunctionType.Relu,
                            bias=bias_t[0:msz, mc:mc + 1])

                    # paint strips: per (batch, colgroup): broadcast copies
                    paint_eng = [nc.vector.tensor_copy,
                                 nc.gpsimd.tensor_copy,
                                 nc.scalar.copy]
                    ei = 0
                    for bi, (gb, npat, groups) in enumerate(degs):
                        base = sum(STRIP * len(g3[2]) for g3 in degs[:bi])
                        for gi, (off, ln) in enumerate(groups):
                            s0 = base + gi * STRIP
                            src01 = t2e[0:128, 0:2, s0:s0 + STRIP]
                            src2 = t2e[0:64, 2, s0:s0 + STRIP]
                            d01 = ob01[0:128, bi, :, :].rearrange(
                                "p m (r c) -> p m r c", r=STRIP)
                            d2 = ob2[0:64, bi, :].rearrange(
                                "p (r c) -> p r c", r=STRIP)
                            cp = paint_eng[ei % 3]
                            ei += 1
                            cp2 = paint_eng[ei % 3]
                            ei += 1
                            if ln == 1:
                                cp(d01[:, :, :, off], src01)
                                cp2(d2[:, :, off], src2)
                            else:
                                # loop order [p, m, len, row] so the source's
                                # last dim is packed (2x/4x DVE modes)
                                dstv = d01[:, :, :, off:off + ln].transpose(
                                    [0, 1, 3, 2])
                                srcv = src01.unsqueeze(2).broadcast_to(
                                    [128, 2, ln, STRIP])
                                cp(dstv, srcv)
                                dstv2 = d2[:, :, off:off + ln].transpose(
                                    [0, 2, 1])
                                srcv2 = src2.unsqueeze(1).broadcast_to(
                                    [64, ln, STRIP])
                                cp2(dstv2, srcv2)

                # ---------- dense conv ----------
                for di in range(ndense):
                    smp = smps[di]
                    for mc, (moff, msz) in enumerate(MB):
                        ps_c = psB.tile([128, 512], f32,
                                        name=f"psc{di}_{mc}", tag="psB")
                        k = 0
                        N_MM = 24
                        for kc, (koff, ksz) in enumerate(MB[:2]):
                            for tap in range(9):
                                dy, dx = tap // 3, tap % 3
                                nc.tensor.matmul(
                                    ps_c[0:msz, :],
                                    wl[0:ksz,
                                       kc * 9 * C + tap * C + moff:
                                       kc * 9 * C + tap * C + moff + msz],
                                    smp[0:ksz, kc, dy:dy + 8, dx:dx + 64],
                                    start=(k == 0), stop=(k == N_MM - 1))
                                k += 1
                        for dx in range(3):
                            # paired kc=2 taps dy=0 (parts 0:64) and dy=1
                            # (parts 64:128, shifted duplicate)
                            nc.tensor.matmul(
                                ps_c[0:msz, :],
                                wl2[0:128, dx * C + moff:
                                    dx * C + moff + msz],
                                smp[0:128, 2, 0:8, dx:dx + 64],
                                start=False, stop=(k == N_MM - 1))
                            k += 1
                            # single kc=2 tap dy=2
                            nc.tensor.matmul(
                                ps_c[0:msz, :],
                                wl[0:64,
                                   2 * 9 * C + (6 + dx) * C + moff:
                                   2 * 9 * C + (6 + dx) * C + moff + msz],
                                smp[0:64, 2, 2:10, dx:dx + 64],
                                start=False, stop=(k == N_MM - 1))
                            k += 1
                        if mc < 2:
                            dst = ob01[0:msz, ndeg + di, mc, :]
                        else:
                            dst = ob2[0:msz, ndeg + di, :]
                        if mc == 1:
                            nc.vector.tensor_scalar(
                                dst, ps_c[0:msz, :],
                                bias_t[0:msz, mc:mc + 1], 0.0,
                                mybir.AluOpType.add, mybir.AluOpType.max)
                        else:
                            nc.scalar.activation(
                                dst, ps_c[0:msz, :],
                                mybir.ActivationFunctionType.Relu,
                                bias=bias_t[0:msz, mc:mc + 1])

                # ---------- output DMAs ----------
                nc.sync.dma_start(out=ob01_d[:, :], in_=ob01[:, :, :, :])
                nc.sync.dma_start(out=ob2_d[:, :], in_=ob2[:, :, :])

            if reps < 0:
                # unrolled, no hardware loop (TimelineSim profiling)
                for u in range(-reps):
                    body(u)
            elif reps == 1:
                body(0)
            else:
                U = 4
                n_loop = reps // U
                hints = (mybir.EngineType.PE, mybir.EngineType.Activation,
                         mybir.EngineType.Pool, mybir.EngineType.SP,
                         mybir.EngineType.DVE)
                with tc.For_i(0, n_loop, 1, hint_engines=hints) as it:
                    for u in range(U):
                        body(u)
                for u in range(reps - n_loop * U):
                    body(u)

    nc.finalize()
    _NC_CACHE[key] = nc
    return nc


# ---------------------------------------------------------------- interface


def make_in_maps(x, source_intrinsics, target_intrinsics, source_pose,
                 target_pose, conv_w, conv_b):
    return make_plan(x, source_intrinsics, target_intrinsics, source_pose,
                     target_pose, conv_w, conv_b)


def assemble(results, slots):
    """results: per-core {"ob01": [128, NB*2*SW], "ob2": [64, NB*SW]}."""
    out = np.zeros((B, C, H, W), dtype=np.float32)
    NBl = len(slots)
    for r in range(NCORE):
        o01 = np.asarray(results[r]["ob01"]).reshape(128, NBl, 2, STRIP, W)
        o2 = np.asarray(results[r]["ob2"]).reshape(64, NBl, STRIP, W)
        for si, gb in enumerate(slots):
            out[gb, 0:128, 8 * r: 8 * r + STRIP, :] = \
                o01[:, si, 0].astype(np.float32)
            out[gb, 128:256, 8 * r: 8 * r + STRIP, :] = \
                o01[:, si, 1].astype(np.float32)
            out[gb, 256:320, 8 * r: 8 * r + STRIP, :] = \
                o2[:, si].astype(np.float32)
    return out


def kernel(x, source_intrinsics, target_intrinsics, source_pose,
           target_pose, conv_w, conv_b, _reps=1):
    from concourse.bass_utils import run_bass_kernel_spmd
    in_maps, struct, slots = make_in_maps(
        x, source_intrinsics, target_intrinsics, source_pose,
        target_pose, conv_w, conv_b)
    nc = build_program(_reps, struct)
    res = run_bass_kernel_spmd(nc, in_maps, list(range(NCORE)))
    return assemble(res.results, slots)


# revision 7
# speedup vs baseline: 1.7628x; 1.2415x over previous
"""Trainium2 Bass kernel for nn_EpipolarWarpOperator (B=8, C=320, H=W=64, S=3).

Sharding: spatial — every core computes an 8-row horizontal strip of all 8
batch outputs (the 3x3 conv is strip-separable; epipolar sampling per strip
with a 1-row halo).

Host analysis classifies each batch by its epipolar sampling map:

* pattern batches: the per-pixel bilinear sampling signature map has few
  distinct values, so the sampled image is piecewise constant and the conv
  output has <= npat distinct "pattern" columns whose column-group structure
  ([edge | interior | edge] per row) is fixed across rows and cores. The
  device computes R[k, tap, pi] from a host-gathered slab via tiny matmuls,
  T[pi, ch] = sum R_tap^T W_tap, then a per-core permutation matmul
  T2E[ch, slot] (slot = (batch, colgroup, striprow)) with relu+bias on the
  drain, and finally paints the output strip with a handful of broadcast
  copies on DVE/Pool/ACT (no per-pixel PE work).

* dense batches: the whole image touches only a few hundred distinct bilinear
  corner pixels; the host gathers them into <=NSL global slabs [128, C] plus
  per-window sampling matrices; the device runs swapped-operand matmuls
  accumulating all slabs into PSUM (two windows: 8 rows + 2 halo rows),
  then a 24-matmul-per-mc 3x3 conv over the 10-row sampled plane with
  bias+relu on the drain.
"""

import numpy as np

B, C, H, W = 8, 320, 64, 64
SN = 3
HW = H * W
STRIP = 8             # output rows per core
NCORE = 8
MB = [(0, 128), (128, 128), (256, 64)]   # output/input channel chunking
PI_MAX = 126          # total pattern budget
NPAT_MAX = 56         # per-batch pattern cap
NSIG_MAX = 64
SW = STRIP * W        # 512 px per strip

# ---------------------------------------------------------------- host: geometry


def _rodrigues_np(rv):
    theta = np.sqrt((rv * rv).sum())
    r = rv / max(theta, 1e-12)
    I = np.eye(3, dtype=np.float32)
    K = np.array([[0, -r[2], r[1]], [r[2], 0, -r[0]], [-r[1], r[0], 0]],
                 dtype=np.float32)
    R = np.cos(theta) * I + (1 - np.cos(theta)) * np.outer(r, r) + np.sin(theta) * K
    return I if theta < 1e-6 else R


def fundamental_np(Ks, Kt, ps, pt):
    Fs = []
    for b in range(Ks.shape[0]):
        Rs = _rodrigues_np(ps[b, :3].astype(np.float32))
        Rt = _rodrigues_np(pt[b, :3].astype(np.float32))
        ts_, tt_ = ps[b, 3:].astype(np.float32), pt[b, 3:].astype(np.float32)
        R_rel = Rs @ Rt.T
        t_rel = ts_ - R_rel @ tt_
        z = np.float32(0)
        skew = np.array([[z, -t_rel[2], t_rel[1]],
                         [t_rel[2], z, -t_rel[0]],
                         [-t_rel[1], t_rel[0], z]], dtype=np.float32)
        E = skew @ R_rel
        inv_Ks = np.linalg.inv(Ks[b].astype(np.float32))
        inv_Kt = np.linalg.inv(Kt[b].astype(np.float32))
        Fs.append(inv_Kt.T @ E @ inv_Ks)
    return np.stack(Fs).astype(np.float32)


def geometry(F):
    k = np.arange(HW)
    px = (k % W).astype(np.float32)
    py = (k // W).astype(np.float32)
    P = np.stack([px, py, np.ones_like(px)])
    lines = F.T.astype(np.float32) @ P
    a, b_, c = lines[0], lines[1], lines[2]
    W1, H1 = np.float32(W - 1), np.float32(H - 1)
    EPS = np.float32(1e-10)
    x1 = np.clip(-c / (a + EPS), 0.0, W1)
    x2 = np.clip(-(b_ * H1 + c) / (a + EPS), 0.0, W1)
    y1 = np.clip(-c / (b_ + EPS), 0.0, H1)
    y2 = np.clip(-(a * W1 + c) / (b_ + EPS), 0.0, H1)
    t = np.linspace(0.0, 1.0, SN, dtype=np.float32)
    sx = x1[:, None] * (1 - t) + x2[:, None] * t
    sy = y1[:, None] * (1 - t) + y2[:, None] * t
    x0 = np.floor(sx)
    y0 = np.floor(sy)
    wx = (sx - x0).astype(np.float32)
    wy = (sy - y0).astype(np.float32)
    x0i = np.clip(x0, 0, W - 1).astype(np.int32)
    y0i = np.clip(y0, 0, H - 1).astype(np.int32)
    return x0i, y0i, wx, wy


def _corners(geo, p, s):
    """4 bilinear (source pixel row, weight/3) pairs for pixel p, sample s."""
    x0i, y0i, wx, wy = geo
    y0 = int(y0i[p, s]); x0 = int(x0i[p, s])
    x1 = min(x0 + 1, W - 1); y1 = min(y0 + 1, H - 1)
    wxx = np.float32(wx[p, s]); wyy = np.float32(wy[p, s])
    third = np.float32(1.0 / 3.0)
    out = {}
    for ry, rx, ww in ((y0, x0, (1 - wxx) * (1 - wyy)),
                       (y0, x1, wxx * (1 - wyy)),
                       (y1, x0, (1 - wxx) * wyy),
                       (y1, x1, wxx * wyy)):
        rr = ry * W + rx
        out[rr] = out.get(rr, np.float32(0)) + ww * third
    return out


# ------------------------------------------------------------- host: classify


def classify(geo):
    x0i, y0i, wx, wy = geo
    key = np.concatenate([
        y0i.astype(np.float32), x0i.astype(np.float32), wx, wy], axis=1)
    kview = np.ascontiguousarray(key).view([('', key.dtype)] * key.shape[1]).ravel()
    uniq, first, inv = np.unique(kview, return_index=True, return_inverse=True)
    if len(uniq) > NSIG_MAX:
        return None
    tables = []
    for si in range(len(uniq)):
        p = int(first[si])
        contrib = {}
        for s in range(SN):
            for rr, ww in _corners(geo, p, s).items():
                contrib[rr] = contrib.get(rr, np.float32(0)) + ww
        tables.append(sorted(contrib.items()))
    return inv.astype(np.int32), tables


def patterns(siginv):
    """3x3 neighborhood patterns of the signature map (border = -1)."""
    simg = siginv.reshape(H, W)
    pad = np.pad(simg, 1, constant_values=-1)
    neigh = np.stack([pad[dy:dy + H, dx:dx + W].ravel()
                      for dy in range(3) for dx in range(3)], axis=1)
    uniq, pinv = np.unique(neigh, axis=0, return_inverse=True)
    return pinv.astype(np.int32), uniq


def col_groups(pinv):
    """Fixed column-group partition of the pattern map: boundaries where any
    row changes pattern. Returns ((off, len), ...)."""
    pimg = pinv.reshape(H, W)
    bset = set()
    for r in range(H):
        row = pimg[r]
        for c in np.where(row[1:] != row[:-1])[0]:
            bset.add(int(c))
    bounds = sorted(bset)
    groups = []
    start = 0
    for b in bounds:
        groups.append((start, b + 1 - start))
        start = b + 1
    groups.append((start, W - start))
    return tuple(groups)


# ------------------------------------------------------------ host: build plan


def _deg_slabs(entries):
    """Assign signatures to slabs so each slab's source-pixel union <= 128."""
    slabs = []
    rows, sigs = {}, []
    for bloc, si, tab in entries:
        new = [(bloc, rr) for rr, _ in tab if (bloc, rr) not in rows]
        if len(rows) + len(new) > 128:
            slabs.append((rows, sigs))
            rows, sigs = {}, []
            new = [(bloc, rr) for rr, _ in tab]
        for key in new:
            rows[key] = len(rows)
        sigs.append((bloc, si))
    slabs.append((rows, sigs))
    return slabs


def _dense_sources(geo, r0, r1):
    """Sorted unique corner source rows for dest rows [r0, r1) (clipped)."""
    x0i, y0i = geo[0], geo[1]
    s = set()
    for r in range(max(r0, 0), min(r1, H)):
        for cx in range(W):
            p = r * W + cx
            for sm in range(SN):
                y0, x0 = int(y0i[p, sm]), int(x0i[p, sm])
                x1, y1 = min(x0 + 1, W - 1), min(y0 + 1, H - 1)
                s.add(y0 * W + x0); s.add(y0 * W + x1)
                s.add(y1 * W + x0); s.add(y1 * W + x1)
    return sorted(s)


def make_plan(x, source_intrinsics, target_intrinsics, source_pose,
              target_pose, conv_w, conv_b):
    Fs = fundamental_np(np.asarray(source_intrinsics, np.float32),
                        np.asarray(target_intrinsics, np.float32),
                        np.asarray(source_pose, np.float32),
                        np.asarray(target_pose, np.float32))
    x = np.asarray(x, np.float32)
    xT16 = [np.ascontiguousarray(x[b].reshape(C, HW).T).astype(np.float16)
            for b in range(B)]

    degs, denses = [], []
    pi_used = 0
    for b in range(B):
        geo = geometry(Fs[b])
        res = classify(geo)
        if res is not None:
            siginv, tables = res
            pinv, ptab = patterns(siginv)
            npat = ptab.shape[0]
            if npat <= NPAT_MAX and pi_used + npat <= PI_MAX:
                degs.append(dict(gb=b, siginv=siginv, tables=tables,
                                 pinv=pinv, ptab=ptab, pi_off=pi_used,
                                 groups=col_groups(pinv)))
                pi_used += npat
                continue
        denses.append(dict(gb=b, geo=geo))
    pi_tot = pi_used

    # ---- degenerate global data (same for all cores) ----
    RGROUP_CAP = 512
    rgroups = []
    cur, cur_n = [], 0
    for bi, d in enumerate(degs):
        n9 = 9 * d['ptab'].shape[0]
        if cur and cur_n + n9 > RGROUP_CAP:
            rgroups.append(dict(bis=cur, ncols=cur_n))
            cur, cur_n = [], 0
        cur.append(bi)
        cur_n += n9
    if cur:
        rgroups.append(dict(bis=cur, ncols=cur_n))

    sg_blocks, sr_blocks = [], []
    for g in rgroups:
        entries = []
        for bloc, bi in enumerate(g['bis']):
            for si, tab in enumerate(degs[bi]['tables']):
                entries.append((bloc, si, tab))
        slabs = _deg_slabs(entries)
        g['nslab'] = len(slabs)
        g['q0'] = degs[g['bis'][0]]['pi_off']
        g['nq'] = sum(degs[bi]['ptab'].shape[0] for bi in g['bis'])
        nq = g['nq']
        for rows, sigs in slabs:
            slab = np.zeros((128, C), dtype=np.float16)
            for (bloc, rr), idx in rows.items():
                slab[idx] = xT16[degs[g['bis'][bloc]]['gb']][rr]
            SR = np.zeros((128, g['ncols']), dtype=np.float32)
            sigset = set(sigs)
            for bloc, bi in enumerate(g['bis']):
                d = degs[bi]
                ptab = d['ptab']
                qb = d['pi_off'] - g['q0']
                for pi in range(ptab.shape[0]):
                    for tap in range(9):
                        si = ptab[pi, tap]
                        if si >= 0 and (bloc, si) in sigset:
                            for rr, ww in d['tables'][si]:
                                SR[rows[(bloc, rr)],
                                   tap * nq + qb + pi] += ww
            sg_blocks.append(slab)
            sr_blocks.append(SR.astype(np.float16))
    sg = (np.concatenate(sg_blocks, axis=1) if sg_blocks
          else np.zeros((128, 0), np.float16))
    sr = (np.concatenate(sr_blocks, axis=1) if sr_blocks
          else np.zeros((128, 0), np.float16))

    # slot layout for the permutation matmul: per deg batch, per colgroup,
    # per strip row (group-major so paint sources are slot-contiguous)
    slot_base = []
    nslot = 0
    for d in degs:
        slot_base.append(nslot)
        nslot += STRIP * len(d['groups'])

    # ---- dense per-core slabs & window sampling matrices ----
    # windows: A = strip rows -1..6 (8 rows, 512 px), B = rows 7..8 (2 rows)
    WINS = ((-1, 8), (7, 2))
    nsl = 1
    core_src = []
    for r in range(NCORE):
        srcs = []
        for d in denses:
            sl = _dense_sources(d['geo'], 8 * r - 1, 8 * r + 9)
            srcs.append(sl)
            nsl = max(nsl, (len(sl) + 127) // 128)
        core_src.append(srcs)
    NSL = nsl

    # ---- weights ----
    Wl = np.zeros((128, 3 * 9 * C), dtype=np.float16)
    cw = np.asarray(conv_w, np.float32)
    cb = np.asarray(conv_b, np.float32)
    for kc, (koff, ksz) in enumerate(MB):
        for tap in range(9):
            dy, dx = tap // 3, tap % 3
            Wl[0:ksz, kc * 9 * C + tap * C: kc * 9 * C + tap * C + C] = \
                cw[:, koff:koff + ksz, dy, dx].T.astype(np.float16)
    # paired kc=2 weights: partitions 0:64 = dy=0 tap, 64:128 = dy=1 tap
    # (read through the +1-row shifted duplicate of the kc=2 sampled plane)
    Wl2 = np.zeros((128, 3 * C), dtype=np.float16)
    for dxi in range(3):
        Wl2[0:64, dxi * C: dxi * C + C] = \
            cw[:, 256:320, 0, dxi].T.astype(np.float16)
        Wl2[64:128, dxi * C: dxi * C + C] = \
            cw[:, 256:320, 1, dxi].T.astype(np.float16)
    bias = np.zeros((128, 3), dtype=np.float32)
    for mc, (moff, msz) in enumerate(MB):
        bias[0:msz, mc] = cb[moff:moff + msz]

    # ---- per-core in_maps ----
    ndeg, ndense = len(degs), len(denses)
    slots = [d['gb'] for d in degs] + [d['gb'] for d in denses]
    in_maps = []
    for r in range(NCORE):
        m = {"wl": Wl, "wl2": Wl2, "bias": bias}
        if ndeg:
            m["sg"] = sg
            m["sr"] = sr
            # permutation matrix: P[pi, slot] with slot = base + g*8 + row
            P = np.zeros((128, nslot), dtype=np.float16)
            for bi, d in enumerate(degs):
                pimg = d['pinv'].reshape(H, W)
                for gi, (off, _ln) in enumerate(d['groups']):
                    for row in range(STRIP):
                        pi = pimg[8 * r + row, off]
                        P[d['pi_off'] + pi,
                          slot_base[bi] + gi * STRIP + row] = 1.0
            m["p"] = P
        if ndense:
            sds, sss = [], []
            for di, d in enumerate(denses):
                sl = core_src[r][di]
                rowmap = {rr: i for i, rr in enumerate(sl)}
                slab = np.zeros((128 * NSL, C), dtype=np.float16)
                if sl:
                    slab[:len(sl)] = xT16[d['gb']][np.array(sl)]
                sds.append(slab.reshape(NSL, 128, C).transpose(1, 0, 2)
                           .reshape(128, NSL * C))
                Sw = []
                for w0, wn in WINS:
                    Smat = np.zeros((128 * NSL, wn * W), dtype=np.float32)
                    for lr in range(wn):
                        row = 8 * r + w0 + lr
                        if not (0 <= row < H):
                            continue
                        for cx in range(W):
                            p = row * W + cx
                            for sm in range(SN):
                                for rr, ww in _corners(d['geo'], p, sm).items():
                                    Smat[rowmap[rr], lr * W + cx] += ww
                    Sw.append(Smat.astype(np.float16))
                # layout: [128, NSL*512 (win A) + NSL*128 (win B)]
                a = Sw[0].reshape(NSL, 128, 8 * W).transpose(1, 0, 2)
                bmat = Sw[1].reshape(NSL, 128, 2 * W).transpose(1, 0, 2)
                sss.append(np.concatenate(
                    [a.reshape(128, -1), bmat.reshape(128, -1)], axis=1))
            m["sd"] = np.concatenate(sds, axis=1)
            m["ss"] = np.concatenate(sss, axis=1)
        in_maps.append(m)

    struct = (pi_tot, nslot, NSL,
              tuple((d['gb'], d['ptab'].shape[0], d['groups'])
                    for d in degs),
              tuple((g['ncols'], g['nslab'], g['q0'], g['nq'])
                    for g in rgroups),
              tuple(d['gb'] for d in denses))
    return in_maps, struct, slots


# ------------------------------------------------------------- bass program

_NC_CACHE = {}


def build_program(reps, struct):
    import os as _os
    abl = _os.environ.get("K_ABL", "")
    key = (reps, struct, abl)
    if key in _NC_CACHE:
        return _NC_CACHE[key]
    import concourse.bacc as bacc
    import concourse.mybir as mybir
    from concourse.tile import TileContext

    fp16 = mybir.dt.float16
    f32 = mybir.dt.float32
    pi_tot, nslot, NSL, degs, rgroups, dense_gbs = struct
    ndeg, ndense = len(degs), len(dense_gbs)
    NB = ndeg + ndense
    NSG = sum(ns for _, ns, _, _ in rgroups)
    SRC = sum(nc_ * ns for nc_, ns, _, _ in rgroups)

    nc = bacc.Bacc(target_bir_lowering=False)
    wl_d = nc.dram_tensor("wl", [128, 3 * 9 * C], fp16, kind="ExternalInput")
    wl2_d = nc.dram_tensor("wl2", [128, 3 * C], fp16, kind="ExternalInput")
    bias_d = nc.dram_tensor("bias", [128, 3], f32, kind="ExternalInput")
    if ndeg:
        sg_d = nc.dram_tensor("sg", [128, NSG * C], fp16, kind="ExternalInput")
        sr_d = nc.dram_tensor("sr", [128, SRC], fp16, kind="ExternalInput")
        p_d = nc.dram_tensor("p", [128, nslot], fp16, kind="ExternalInput")
    if ndense:
        sd_d = nc.dram_tensor("sd", [128, ndense * NSL * C], fp16,
                              kind="ExternalInput")
        ss_d = nc.dram_tensor("ss", [128, ndense * NSL * 640], fp16,
                              kind="ExternalInput")
    ob01_d = nc.dram_tensor("ob01", [128, NB * 2 * SW], fp16,
                            kind="ExternalOutput")
    ob2_d = nc.dram_tensor("ob2", [64, NB * SW], fp16, kind="ExternalOutput")

    with TileContext(nc) as tc:
        with tc.tile_pool(name="const", bufs=1) as constp, \
             tc.tile_pool(name="inp", bufs=2) as inp, \
             tc.tile_pool(name="sdp", bufs=2) as sdp, \
             tc.tile_pool(name="work", bufs=2) as work, \
             tc.tile_pool(name="smpp", bufs=2) as smpp, \
             tc.tile_pool(name="outp", bufs=2) as outp, \
             tc.tile_pool(name="psA", bufs=4, space="PSUM") as psA, \
             tc.tile_pool(name="psB", bufs=4, space="PSUM") as psB:
            wl = constp.tile([128, 3 * 9 * C], fp16)
            nc.sync.dma_start(out=wl[:], in_=wl_d[:])
            wl2 = constp.tile([128, 3 * C], fp16)
            nc.sync.dma_start(out=wl2[:], in_=wl2_d[:])
            bias_t = constp.tile([128, 3], f32)
            nc.sync.dma_start(out=bias_t[:], in_=bias_d[:])

            def body(_it):
                ob01 = outp.tile([128, NB, 2, SW], fp16, name="ob01",
                                 tag="ob01")
                ob2 = outp.tile([64, NB, SW], fp16, name="ob2", tag="ob2")

                # ---------- input DMAs ----------
                if ndeg:
                    sg = inp.tile([128, NSG * C], fp16, name="sg", tag="sg")
                    nc.sync.dma_start(out=sg[:], in_=sg_d[:])
                    sr = inp.tile([128, SRC], fp16, name="sr", tag="sr")
                    nc.sync.dma_start(out=sr[:], in_=sr_d[:])
                    p_sb = inp.tile([128, nslot], fp16, name="p", tag="p")
                    nc.sync.dma_start(out=p_sb[0:pi_tot, :],
                                      in_=p_d[0:pi_tot, :])
                if ndense:
                    sd = sdp.tile([128, ndense * NSL * C], fp16, name="sd",
                                  tag="sd")
                    nc.sync.dma_start(out=sd[:], in_=sd_d[:])
                    ss = sdp.tile([128, ndense * NSL * 640], fp16, name="ss",
                                  tag="ss")
                    nc.sync.dma_start(out=ss[:], in_=ss_d[:])

                # ---------- degenerate path: R ----------
                if ndeg:
                    rsb = work.tile([128, 3, 9, pi_tot], fp16, name="rsb",
                                    tag="rsb")
                    sgo, sro = 0, 0
                    for gi, (ncols, nsl, q0, nq) in enumerate(rgroups):
                        for kc, (koff, ksz) in enumerate(MB):
                            ps_r = psB.tile([128, 512], f32,
                                            name=f"psr{gi}_{kc}", tag="psB")
                            for j in range(nsl):
                                nc.tensor.matmul(
                                    ps_r[0:ksz, 0:ncols],
                                    sg[:, (sgo + j) * C + koff:
                                       (sgo + j) * C + koff + ksz],
                                    sr[:, sro + j * ncols:
                                       sro + (j + 1) * ncols],
                                    start=(j == 0), stop=(j == nsl - 1))
                            psq = ps_r[:, 0:ncols].rearrange(
                                "p (t q) -> p t q", t=9)
                            if kc % 2 == 0:
                                nc.vector.tensor_copy(
                                    rsb[0:ksz, kc, :, q0:q0 + nq], psq[0:ksz])
                            else:
                                nc.scalar.copy(
                                    rsb[0:ksz, kc, :, q0:q0 + nq], psq[0:ksz])
                        sgo += nsl
                        sro += nsl * ncols

                # ---------- dense sampling ----------
                smps = []
                if ndense and 'nosamp' not in abl:
                    for di in range(ndense):
                        # plane row i = image row 8r-1+i; cols 1..64 = image
                        smp = smpp.tile([128, 3, 10, 66], fp16,
                                        name=f"smp{di}", tag=f"smp{di}")
                        nc.gpsimd.memset(smp[:, :, :, 0:1], 0.0)
                        nc.gpsimd.memset(smp[:, :, :, 65:66], 0.0)
                        smps.append(smp)

                    for di in range(ndense):
                        smp = smps[di]
                        sdo = di * NSL * C
                        sso = di * NSL * 640
                        # window A: 8 rows, 512 px
                        pws = []
                        for kc, (koff, ksz) in enumerate(MB):
                            pw = psA.tile([128, 512], f32,
                                          name=f"pswA{di}_{kc}", tag="psA")
                            for j in range(NSL):
                                nc.tensor.matmul(
                                    pw[0:ksz, :],
                                    sd[:, sdo + j * C + koff:
                                       sdo + j * C + koff + ksz],
                                    ss[:, sso + j * 512:sso + (j + 1) * 512],
                                    start=(j == 0), stop=(j == NSL - 1))
                            pws.append(pw)
                        # window B: 2 rows, 128 px (3 kc share one bank)
                        pwb = psA.tile([128, 512], f32, name=f"pswB{di}",
                                       tag="psA")
                        for kc, (koff, ksz) in enumerate(MB):
                            for j in range(NSL):
                                nc.tensor.matmul(
                                    pwb[0:ksz, kc * 128:(kc + 1) * 128],
                                    sd[:, sdo + j * C + koff:
                                       sdo + j * C + koff + ksz],
                                    ss[:, sso + NSL * 512 + j * 128:
                                       sso + NSL * 512 + (j + 1) * 128],
                                    start=(j == 0), stop=(j == NSL - 1))
                        # drains: win A rows 0..7, win B rows 8..9
                        for kc, (koff, ksz) in enumerate(MB):
                            pv = pws[kc][:, :].rearrange("p (r c) -> p r c",
                                                         r=8)
                            if kc == 1:
                                nc.scalar.copy(smp[0:ksz, kc, 0:8, 1:65],
                                               pv[0:ksz])
                            else:
                                nc.vector.tensor_copy(
                                    smp[0:ksz, kc, 0:8, 1:65], pv[0:ksz])
                        pvb = pwb[:, 0:384].rearrange("p (k r c) -> p k r c",
                                                      k=3, r=2)
                        nc.vector.tensor_copy(smp[0:128, 0:2, 8:10, 1:65],
                                              pvb[0:128, 0:2])
                        nc.scalar.copy(smp[0:64, 2, 8:10, 1:65],
                                       pvb[0:64, 2])
                        # +1-row shifted duplicate of kc=2 plane in parts
                        # 64:128 (for the paired dy0/dy1 conv taps)
                        nc.gpsimd.tensor_copy(smp[64:128, 2, 0:9, 1:65],
                                               smp[0:64, 2, 1:10, 1:65])

                # ---------- degenerate path: T, perm, paint ----------
                if ndeg and 'not2' not in abl:
                    ps_t = psB.tile([128, 512], f32, name="ps_t", tag="psB")
                    k = 0
                    for kc, (koff, ksz) in enumerate(MB):
                        for tap in range(9):
                            nc.tensor.matmul(
                                ps_t[0:pi_tot, 0:C],
                                rsb[0:ksz, kc, tap, :],
                                wl[0:ksz, kc * 9 * C + tap * C:
                                   kc * 9 * C + tap * C + C],
                                start=(k == 0), stop=(k == 26))
                            k += 1
                    tsb = work.tile([128, C], fp16, name="tsb", tag="tsb")
                    nc.scalar.copy(tsb[0:pi_tot, :], ps_t[0:pi_tot, 0:C])

                    t2e = work.tile([128, 3, nslot], fp16, name="t2e",
                                    tag="t2e")
                    for mc, (moff, msz) in enumerate(MB):
                        ps_e = psB.tile([128, 512], f32, name=f"pse{mc}",
                                        tag="psB")
                        nc.tensor.matmul(
                            ps_e[0:msz, 0:nslot],
                            tsb[0:pi_tot, moff:moff + msz],
                            p_sb[0:pi_tot, :],
                            start=True, stop=True)
                        nc.scalar.activation(
                            t2e[0:msz, mc, :], ps_e[0:msz, 0:nslot],
                            mybir.ActivationFunctionType.Relu,
                            bias=bias_t[0:msz, mc:mc + 1])

                    # paint strips: per (batch, colgroup): broadcast copies
                    do_paint = 'nopaint' not in abl
                    paint_eng = [nc.vector.tensor_copy,
                                 nc.gpsimd.tensor_copy,
                                 nc.scalar.copy]
                    ei = 0
                    for bi, (gb, npat, groups) in enumerate(degs):
                        if not do_paint:
                            break
                        base = sum(STRIP * len(g3[2]) for g3 in degs[:bi])
                        for gi, (off, ln) in enumerate(groups):
                            s0 = base + gi * STRIP
                            src01 = t2e[0:128, 0:2, s0:s0 + STRIP]
                            src2 = t2e[0:64, 2, s0:s0 + STRIP]
                            d01 = ob01[0:128, bi, :, :].rearrange(
                                "p m (r c) -> p m r c", r=STRIP)
                            d2 = ob2[0:64, bi, :].rearrange(
                                "p (r c) -> p r c", r=STRIP)
                            cp = paint_eng[ei % 3]
                            ei += 1
                            cp2 = paint_eng[ei % 3]
                            ei += 1
                            if ln == 1:
                                cp(d01[:, :, :, off], src01)
                                cp2(d2[:, :, off], src2)
                            else:
                                # loop order [p, m, len, row] so the source's
                                # last dim is packed (2x/4x DVE modes)
                                dstv = d01[:, :, :, off:off + ln].transpose(
                                    [0, 1, 3, 2])
                                srcv = src01.unsqueeze(2).broadcast_to(
                                    [128, 2, ln, STRIP])
                                cp(dstv, srcv)
                                dstv2 = d2[:, :, off:off + ln].transpose(
                                    [0, 2, 1])
                                srcv2 = src2.unsqueeze(1).broadcast_to(
                                    [64, ln, STRIP])
                                cp2(dstv2, srcv2)

                # ---------- dense conv ----------
                for di in range(ndense if ('noconv' not in abl and 'nosamp' not in abl) else 0):
                    smp = smps[di]
                    for mc, (moff, msz) in enumerate(MB):
                        ps_c = psB.tile([128, 512], f32,
                                        name=f"psc{di}_{mc}", tag="psB")
                        k = 0
                        N_MM = 24
                        for kc, (koff, ksz) in enumerate(MB[:2]):
                            for tap in range(9):
                                dy, dx = tap // 3, tap % 3
                                nc.tensor.matmul(
                                    ps_c[0:msz, :],
                                    wl[0:ksz,
                                       kc * 9 * C + tap * C + moff:
                                       kc * 9 * C + tap * C + moff + msz],
                                    smp[0:ksz, kc, dy:dy + 8, dx:dx + 64],
                                    start=(k == 0), stop=(k == N_MM - 1))
                                k += 1
                        for dx in range(3):
                            # paired kc=2 taps dy=0 (parts 0:64) and dy=1
                            # (parts 64:128, shifted duplicate)
                            nc.tensor.matmul(
                                ps_c[0:msz, :],
                                wl2[0:128, dx * C + moff:
                                    dx * C + moff + msz],
                                smp[0:128, 2, 0:8, dx:dx + 64],
                                start=False, stop=(k == N_MM - 1))
                            k += 1
                            # single kc=2 tap dy=2
                            nc.tensor.matmul(
                                ps_c[0:msz, :],
                                wl[0:64,
                                   2 * 9 * C + (6 + dx) * C + moff:
                                   2 * 9 * C + (6 + dx) * C + moff + msz],
                                smp[0:64, 2, 2:10, dx:dx + 64],
                                start=False, stop=(k == N_MM - 1))
                            k += 1
                        if mc < 2:
                            dst = ob01[0:msz, ndeg + di, mc, :]
                        else:
                            dst = ob2[0:msz, ndeg + di, :]
                        if mc == 1:
                            nc.vector.tensor_scalar(
                                dst, ps_c[0:msz, :],
                                bias_t[0:msz, mc:mc + 1], 0.0,
                                mybir.AluOpType.add, mybir.AluOpType.max)
                        else:
                            nc.scalar.activation(
                                dst, ps_c[0:msz, :],
                                mybir.ActivationFunctionType.Relu,
                                bias=bias_t[0:msz, mc:mc + 1])

                # ---------- output DMAs ----------
                nc.sync.dma_start(out=ob01_d[:, :], in_=ob01[:, :, :, :])
                nc.sync.dma_start(out=ob2_d[:, :], in_=ob2[:, :, :])

            if reps < 0:
                # unrolled, no hardware loop (TimelineSim profiling)
                for u in range(-reps):
                    body(u)
            elif reps == 1:
                body(0)
            else:
                U = 4
                n_loop = reps // U
                hints = (mybir.EngineType.PE, mybir.EngineType.Activation,
                         mybir.EngineType.Pool, mybir.EngineType.SP,
                         mybir.EngineType.DVE)
                with tc.For_i(0, n_loop, 1, hint_engines=hints) as it:
                    for u in range(U):
                        body(u)
                for u in range(reps - n_loop * U):
                    body(u)

    nc.finalize()
    _NC_CACHE[key] = nc
    return nc


# ---------------------------------------------------------------- interface


def make_in_maps(x, source_intrinsics, target_intrinsics, source_pose,
                 target_pose, conv_w, conv_b):
    return make_plan(x, source_intrinsics, target_intrinsics, source_pose,
                     target_pose, conv_w, conv_b)


def assemble(results, slots):
    """results: per-core {"ob01": [128, NB*2*SW], "ob2": [64, NB*SW]}."""
    out = np.zeros((B, C, H, W), dtype=np.float32)
    NBl = len(slots)
    for r in range(NCORE):
        o01 = np.asarray(results[r]["ob01"]).reshape(128, NBl, 2, STRIP, W)
        o2 = np.asarray(results[r]["ob2"]).reshape(64, NBl, STRIP, W)
        for si, gb in enumerate(slots):
            out[gb, 0:128, 8 * r: 8 * r + STRIP, :] = \
                o01[:, si, 0].astype(np.float32)
            out[gb, 128:256, 8 * r: 8 * r + STRIP, :] = \
                o01[:, si, 1].astype(np.float32)
            out[gb, 256:320, 8 * r: 8 * r + STRIP, :] = \
                o2[:, si].astype(np.float32)
    return out


def kernel(x, source_intrinsics, target_intrinsics, source_pose,
           target_pose, conv_w, conv_b, _reps=1):
    from concourse.bass_utils import run_bass_kernel_spmd
    in_maps, struct, slots = make_in_maps(
        x, source_intrinsics, target_intrinsics, source_pose,
        target_pose, conv_w, conv_b)
    nc = build_program(_reps, struct)
    res = run_bass_kernel_spmd(nc, in_maps, list(range(NCORE)))
    return assemble(res.results, slots)
